# revision 31
# baseline (speedup 1.0000x reference)
"""Trainium2 Bass kernel for AssociativeMemoryModule (causal linear attention).

Sharding: head-parallel with output-partial unshard - core c owns head c for
both batches and NEVER communicates on-device (no collectives, no NRT
pre-collective barrier). Each core:
  1. projects full x (pre-transposed, bf16 on host) to [q.T;k.T] (128 rows)
     and v.T (64 rows); phi = min(exp(z),1) + relu(z) in f32 -> bf16,
  2. stacks [kT; vT] at base partition 0 so one PE transpose per 128-chunk
     yields both k and v in normal layout; masked scores batched per t-tile,
  3. chunked causal linear attention (C=128) computed TRANSPOSED: poT[m,t]
     needs no per-chunk PE transposes or epilogue copies since all operand
     layouts (v_aug [s,m], sm [s,t], Sb16 [d,m], qT [d,t]) already exist.
     The intra-chunk matmuls open PSUM accumulation groups early (filling
     the PE while the serial DVE state-chain runs); the state matmuls close
     them as each prefix state lands. State chain is kept in bf16 (one DVE
     add per chunk, no separate cast).
  4. o-projection flipped: oT[Dc,t] = woh[:,Dc].T @ onT, emitted UNNORMALIZED
     as bf16 [4,128,BT] plus the bf16 denominator row [1,BT] (row 64 of onT);
     the per-token 1/denom division commutes with o-proj and is done on host.
Host unshards: o = sum_h (oT_h / den_h).T + bo.
"""
import sys

import numpy as np

sys.path.insert(0, "/opt/trn_rl_repo")

H, HD, D = 8, 64, 512
B, T = 2, 1024
BT = B * T            # 2048
C = 128               # attention chunk
NCH = BT // C         # 16 chunks total
CPB = T // C          # 8 chunks per batch
NF = D // 128         # 4 feature tiles
NT = 4                # t-tiles of 512 for projections

_CACHE = {}


def _build():
    if "nc" in _CACHE:
        return _CACHE["nc"]
    import concourse.mybir as mybir
    import concourse.tile as tile
    from concourse import bacc
    from concourse.bass import ts

    import ml_dtypes

    f32 = mybir.dt.float32
    bf16 = mybir.dt.bfloat16
    AF = mybir.ActivationFunctionType

    nc = bacc.Bacc("TRN2", target_bir_lowering=False, debug=False, num_devices=8,
                   num_swdge_queues=4)

    xT = nc.declare_dram_parameter("xT", [D, BT], bf16, isOutput=False)
    wa = nc.declare_dram_parameter("wa", [D, 128], bf16, isOutput=False)
    wv = nc.declare_dram_parameter("wv", [D, HD], bf16, isOutput=False)
    woh = nc.declare_dram_parameter("woh", [HD, D], bf16, isOutput=False)
    bqk = nc.declare_dram_parameter("bqk", [128, 1], f32, isOutput=False)
    bv = nc.declare_dram_parameter("bv", [HD, 1], f32, isOutput=False)
    outT = nc.declare_dram_parameter("outT", [NF, 128, BT], bf16, isOutput=True)
    den = nc.declare_dram_parameter("den", [1, BT], bf16, isOutput=True)

    # mask4[s, jj, t] = s <= t (same causal mask for each of 4 chunks)
    mask_np = np.broadcast_to(
        np.triu(np.ones((C, C), np.float32))[:, None, :], (C, 4, C)).copy()
    iden128_np = np.eye(C, dtype=ml_dtypes.bfloat16)
    mask_d = nc.inline_tensor(mask_np.reshape(C, 4 * C), "causal_mask4")
    iden128_d = nc.inline_tensor(iden128_np, "iden128")

    with tile.TileContext(nc) as tc:
        with (
            tc.tile_pool(name="consts", bufs=1) as consts,
        ):
            # ---- resident SBUF tensors (matmul operands in bf16) ----
            xt_sb = consts.tile([128, NF, BT], bf16)
            wa_sb = consts.tile([128, NF, 128], bf16)
            wv_sb = consts.tile([128, NF, HD], bf16)
            woh_sb = consts.tile([HD, D], bf16)
            bqk_sb = consts.tile([128, 1], f32)
            bv_sb = consts.tile([HD, 1], f32)
            mask_sb = consts.tile([C, 4, C], f32)
            iden128_sb = consts.tile([C, C], bf16)
            qk_phi = consts.tile([128, BT], bf16)      # rows 0-63 qT, 64-127 kT
            kvT = consts.tile([128, BT], bf16)         # rows 0-63 kT, 64-127 vT
            vT_sb = consts.tile([HD, BT], bf16)
            k_nrm = consts.tile([128, NCH, HD], bf16)
            v_aug = consts.tile([128, NCH, HD + 1], bf16)
            sm_all = consts.tile([C, NCH, C], bf16)
            Sb16 = consts.tile([HD, B, CPB - 1, HD + 1], bf16)
            onT_all = consts.tile([HD + 1, NCH, C], bf16)
            dumX = consts.tile([128, 512], bf16)

            # ---- input staging. Each dma_start blocks its queue ~600ns, so
            # the per-queue issue ORDER is the schedule: first-needed first.
            # (batched/rearranged weight DMAs explode into tiny descriptors
            # and delay queue-completion counts - keep tiles fine-grained)
            def xt_dma(eng, tcol, f):
                eng.dma_start(xt_sb[:, f, ts(tcol, 512)],
                              xT[128 * f:128 * (f + 1), ts(tcol, 512)])

            nc.sync.dma_start(wa_sb[:, 0, :], wa[0:128, :])
            nc.sync.dma_start(wa_sb[:, 1, :], wa[128:256, :])
            xt_dma(nc.sync, 0, 0)
            xt_dma(nc.sync, 0, 2)
            xt_dma(nc.gpsimd, 0, 1)
            nc.gpsimd.dma_start(wa_sb[:, 2, :], wa[256:384, :])
            nc.gpsimd.dma_start(wa_sb[:, 3, :], wa[384:512, :])
            nc.scalar.dma_start(bqk_sb[:], bqk[:, :])
            xt_dma(nc.scalar, 0, 3)
            nc.scalar.dma_start(bv_sb[:], bv[:, :])
            xt_dma(nc.scalar, 1, 1)
            xt_dma(nc.scalar, 1, 3)
            for f in range(NF):
                nc.gpsimd.dma_start(wv_sb[:, f, :], wv[128 * f:128 * (f + 1), :])
            nc.gpsimd.dma_start(iden128_sb[:], iden128_d[:, :])
            nc.gpsimd.dma_start(mask_sb[:],
                                mask_d.ap().rearrange("p (j t) -> p j t", j=4))
            nc.vector.memset(v_aug[:, :, HD:HD + 1], 1.0)
            nc.vector.memset(dumX[:], 0.0)
            xt_dma(nc.scalar, 2, 1)
            xt_dma(nc.scalar, 2, 3)
            xt_dma(nc.scalar, 3, 1)
            xt_dma(nc.scalar, 3, 3)
            for tcol in range(1, NT):
                xt_dma(nc.sync, tcol, 0)
                xt_dma(nc.sync, tcol, 2)
            nc.sync.dma_start(woh_sb[:], woh[:, :])

            with (
                tc.tile_pool(name="psA", bufs=3, space="PSUM") as psA,
                tc.tile_pool(name="psSc", bufs=2, space="PSUM") as psSc,
                tc.tile_pool(name="psT", bufs=3, space="PSUM") as psT,
                tc.tile_pool(name="ptmp", bufs=2) as ptmp,
                tc.tile_pool(name="attn", bufs=4) as attn,
            ):
                def proj_tile(tt):
                    sl = ts(tt, 512)
                    pa = psA.tile([128, 512], f32, tag="pa", name=f"pa{tt}")
                    pb = psA.tile([HD, 512], f32, tag="pa", name=f"pb{tt}")
                    for f in range(NF):
                        nc.tensor.matmul(pa, wa_sb[:, f, :], xt_sb[:, f, sl],
                                         start=(f == 0), stop=(f == NF - 1))
                        if tt <= 1:
                            filler(1)
                    for f in range(NF):
                        nc.tensor.matmul(pb, wv_sb[:, f, :], xt_sb[:, f, sl],
                                         start=(f == 0), stop=(f == NF - 1))
                        if tt <= 1:
                            filler(1)
                    if tt <= 1:
                        filler(2)
                    nc.scalar.activation(vT_sb[:, sl], pb, AF.Identity, bias=bv_sb[:])
                    # phi = exp(min(z,0)) + relu(z) = min(exp(z),1) + relu(z):
                    # both ACT ops read PSUM directly with fused bias
                    rr = ptmp.tile([128, 512], f32, tag="rr", name=f"rr{tt}")
                    ee = ptmp.tile([128, 512], f32, tag="ee", name=f"ee{tt}")
                    nc.scalar.activation(ee, pa, AF.Exp, bias=bqk_sb[:])
                    nc.scalar.activation(rr, pa, AF.Relu, bias=bqk_sb[:])
                    nc.vector.scalar_tensor_tensor(
                        qk_phi[:, sl], ee, 1.0, rr,
                        mybir.AluOpType.min, mybir.AluOpType.add)
                    # build [kT; vT] at base partition 0 (SBUF->SBUF DMAs):
                    # matmul operands must share a base partition, and the
                    # stacked tile transposes k and v chunks in ONE PE op
                    nc.sync.dma_start(kvT[0:HD, sl], qk_phi[64:128, sl])
                    nc.gpsimd.dma_start(kvT[HD:128, sl], vT_sb[:, sl])
                    # transposes + scores for the 4 chunks in this t-tile
                    filler(2)
                    ptr = psSc.tile([C, 4, C], bf16, tag="ps", name=f"tr{tt}")
                    psc = psSc.tile([C, 4, C], f32, tag="ps", name=f"ps{tt}")
                    for jj in range(4):
                        i = tt * 4 + jj
                        cs = ts(i, C)
                        nc.tensor.transpose(ptr[:, jj, :], kvT[:, cs],
                                            iden128_sb[:])
                        nc.tensor.matmul(psc[:, jj, :], kvT[0:HD, cs],
                                         qk_phi[0:64, cs], start=True, stop=True)
                    i0 = tt * 4
                    nc.vector.tensor_copy(k_nrm[:, i0:i0 + 4, :], ptr[:, :, 0:HD])
                    nc.vector.tensor_copy(v_aug[:, i0:i0 + 4, 0:HD],
                                          ptr[:, :, HD:2 * HD])
                    nc.vector.tensor_mul(sm_all[:, i0:i0 + 4, :], psc, mask_sb[:])

                def kv_part(b):
                    # kv outer products (PE) + bf16 prefix-state chain (DVE):
                    # Sb16[j] = Sb16[j-1] + pkv[j], j = 0..CPB-2
                    for w in range(2):
                        pkv = psT.tile([HD, 4, HD + 1], f32, tag="tr",
                                       name=f"pkv{b}{w}")
                        for jw in range(4 if w == 0 else 3):
                            j = 4 * w + jw
                            i = b * CPB + j
                            nc.tensor.matmul(pkv[:, jw, :], k_nrm[:, i, :],
                                             v_aug[:, i, :], start=True, stop=True)
                        for jw in range(4 if w == 0 else 3):
                            j = 4 * w + jw
                            if j == 0:
                                nc.vector.tensor_copy(Sb16[:, b, 0, :],
                                                      pkv[:, 0, :])
                            else:
                                nc.vector.tensor_add(Sb16[:, b, j, :],
                                                     Sb16[:, b, j - 1, :],
                                                     pkv[:, jw, :])

                poT = {}

                def po_pair(b, j):
                    # one chunk: intra + state matmuls ADJACENT (a start=True
                    # clears has_written for the whole bank, so a chunk's
                    # group must close before the next group in that bank
                    # opens). Chains of b=0 / b=1 are independent, so pairs
                    # are interleaved across batches to hide chain latency.
                    g = 2 * b + j // 4
                    jw = j % 4
                    i = b * CPB + j
                    if jw == 0:
                        poT[g] = psA.tile([HD + 1, 4, C], f32, tag="pa",
                                          name=f"poT{g}")
                    p = poT[g]
                    if j == 0:
                        nc.tensor.matmul(p[:, 0, :], v_aug[:, i, :],
                                         sm_all[:, i, :], start=True, stop=True)
                    else:
                        nc.tensor.matmul(p[:, jw, :], v_aug[:, i, :],
                                         sm_all[:, i, :], start=True, stop=False)
                        nc.tensor.matmul(p[:, jw, :], Sb16[:, b, j - 1, :],
                                         qk_phi[0:64, ts(i, C)],
                                         start=False, stop=True)

                def onT_copy(g):
                    b, w = g // 2, g % 2
                    i0 = b * CPB + 4 * w
                    if g >= 2:
                        nc.vector.tensor_copy(onT_all[:, i0:i0 + 4, :],
                                              poT[g][:])
                    else:
                        nc.scalar.copy(onT_all[:, i0:i0 + 4, :], poT[g][:])

                def o_proj_mm(g, Dc):
                    i0 = (g // 2) * CPB + 4 * (g % 2)
                    ppT = psSc.tile([C, 512], f32, tag="ps", name=f"ppT{g}{Dc}")
                    nc.tensor.matmul(ppT, woh_sb[:, ts(Dc, 128)],
                                     onT_all[0:HD, i0:i0 + 4, :],
                                     start=True, stop=True)
                    osl = attn.tile([C, 512], bf16, tag="osl",
                                    name=f"osl{g}{Dc}")
                    # split the PSUM->SBUF bounce across both PSUM-capable
                    # engines so the pool slot frees in ~one half-copy; in
                    # the pairs phase (g0/g2) Vector is the pacer, so Scalar
                    # takes the whole tile there
                    if g == 0:
                        nc.scalar.copy(osl[:], ppT[:])
                    else:
                        nc.scalar.copy(osl[:, 0:224], ppT[:, 0:224])
                        nc.vector.tensor_copy(osl[:, 224:512],
                                              ppT[:, 224:512])
                    eng = (nc.sync, nc.gpsimd)[Dc % 2]
                    eng.dma_start(outT[Dc, :, ts(g, 512)], osl)

                dps = psA.tile([128, 512], f32, tag="pa", name="warm")
                dps2 = psT.tile([128, 512], f32, tag="tr", name="warm2")

                def filler(n):
                    # short always-ready matmuls: bridge dependency gaps so
                    # the PE p-state governor never sees a >150ns idle and
                    # keeps the array at max clock
                    for _ in range(n):
                        nc.tensor.matmul(dps2[:, 0:128], dumX[:, 0:128],
                                         dumX[:, 0:128], start=True, stop=True)

                for wi in range(14):
                    nc.tensor.matmul(dps[:, 0:256], dumX[:, 0:128],
                                     dumX[:, 0:256], start=True, stop=True)
                proj_tile(0)
                proj_tile(1)
                proj_tile(2)
                proj_tile(3)
                kv_part(0)
                kv_part(1)
                po_pair(0, 0)
                po_pair(0, 1)
                po_pair(0, 2)
                po_pair(0, 3)
                onT_copy(0)
                po_pair(0, 4)
                po_pair(0, 5)
                o_proj_mm(0, 0)
                o_proj_mm(0, 1)
                po_pair(0, 6)
                po_pair(0, 7)
                onT_copy(1)
                o_proj_mm(0, 2)
                o_proj_mm(0, 3)
                po_pair(1, 0)
                po_pair(1, 1)
                o_proj_mm(1, 0)
                o_proj_mm(1, 1)
                po_pair(1, 2)
                po_pair(1, 3)
                onT_copy(2)
                o_proj_mm(1, 2)
                o_proj_mm(1, 3)
                po_pair(1, 4)
                po_pair(1, 5)
                o_proj_mm(2, 0)
                o_proj_mm(2, 1)
                po_pair(1, 6)
                po_pair(1, 7)
                onT_copy(3)
                o_proj_mm(2, 2)
                o_proj_mm(2, 3)
                nc.sync.dma_start(den[0:1, :], onT_all[HD:HD + 1, :, :])

                o_proj_mm(3, 0)
                o_proj_mm(3, 1)
                o_proj_mm(3, 2)
                o_proj_mm(3, 3)
    nc.compile()
    _CACHE["nc"] = nc
    return nc


def _in_maps(x, Wq, bq, Wk, bk, Wv, bv, Wo, bo):
    import ml_dtypes
    bf = ml_dtypes.bfloat16
    x2 = np.ascontiguousarray(x.reshape(BT, D).T).astype(bf)
    WoT = np.ascontiguousarray(Wo.T)                  # [(h m), d]
    maps = []
    for c in range(8):
        sl = slice(HD * c, HD * (c + 1))
        maps.append(dict(
            xT=x2,
            wa=np.ascontiguousarray(np.concatenate([Wq[sl], Wk[sl]], 0).T).astype(bf),
            wv=np.ascontiguousarray(Wv[sl].T).astype(bf),
            woh=np.ascontiguousarray(WoT[sl]).astype(bf),
            bqk=np.ascontiguousarray(np.concatenate([bq[sl], bk[sl]]).reshape(128, 1)).astype(np.float32),
            bv=np.ascontiguousarray(bv[sl].reshape(HD, 1)).astype(np.float32),
        ))
    return maps


def kernel(x, Wq, bq, Wk, bk, Wv, bv, Wo, bo):
    from concourse import bass_utils

    nc = _build()
    maps = _in_maps(np.asarray(x), np.asarray(Wq), np.asarray(bq),
                    np.asarray(Wk), np.asarray(bk), np.asarray(Wv),
                    np.asarray(bv), np.asarray(Wo), np.asarray(bo))
    res = bass_utils.run_bass_kernel_spmd(nc, maps, core_ids=list(range(8)))
    # unshard: per-head UNNORMALIZED partials oT [D, BT] + denom [1, BT];
    # divide by each head's denominator, sum over heads, add bias
    accT = np.zeros((D, BT), np.float32)
    for c in range(8):
        oT = res.results[c]["outT"].astype(np.float32).reshape(D, BT)
        dn = np.maximum(res.results[c]["den"].astype(np.float32), 1e-6)
        accT += oT / dn
    o = accT.T.reshape(B, T, D) + np.asarray(bo).astype(np.float32)[None, None, :]
    return np.ascontiguousarray(o).astype(np.float32)


# revision 32
# speedup vs baseline: 1.0019x; 1.0019x over previous
"""Trainium2 Bass kernel for AssociativeMemoryModule (causal linear attention).

Sharding: head-parallel with output-partial unshard - core c owns head c for
both batches and NEVER communicates on-device (no collectives, no NRT
pre-collective barrier). Each core:
  1. projects full x (pre-transposed, bf16 on host) to [q.T;k.T] (128 rows)
     and v.T (64 rows); phi = min(exp(z),1) + relu(z) in f32 -> bf16,
  2. stacks [kT; vT] at base partition 0 so one PE transpose per 128-chunk
     yields both k and v in normal layout; masked scores batched per t-tile,
  3. chunked causal linear attention (C=128) computed TRANSPOSED: poT[m,t]
     needs no per-chunk PE transposes or epilogue copies since all operand
     layouts (v_aug [s,m], sm [s,t], Sb16 [d,m], qT [d,t]) already exist.
     The intra-chunk matmuls open PSUM accumulation groups early (filling
     the PE while the serial DVE state-chain runs); the state matmuls close
     them as each prefix state lands. State chain is kept in bf16 (one DVE
     add per chunk, no separate cast).
  4. o-projection flipped: oT[Dc,t] = woh[:,Dc].T @ onT, emitted UNNORMALIZED
     as bf16 [4,128,BT] plus the bf16 denominator row [1,BT] (row 64 of onT);
     the per-token 1/denom division commutes with o-proj and is done on host.
Host unshards: o = sum_h (oT_h / den_h).T + bo.
"""
import sys

import numpy as np

sys.path.insert(0, "/opt/trn_rl_repo")

H, HD, D = 8, 64, 512
B, T = 2, 1024
BT = B * T            # 2048
C = 128               # attention chunk
NCH = BT // C         # 16 chunks total
CPB = T // C          # 8 chunks per batch
NF = D // 128         # 4 feature tiles
NT = 4                # t-tiles of 512 for projections

_CACHE = {}


def _build():
    if "nc" in _CACHE:
        return _CACHE["nc"]
    import concourse.mybir as mybir
    import concourse.tile as tile
    from concourse import bacc
    from concourse.bass import ts

    import ml_dtypes

    f32 = mybir.dt.float32
    bf16 = mybir.dt.bfloat16
    AF = mybir.ActivationFunctionType

    nc = bacc.Bacc("TRN2", target_bir_lowering=False, debug=False, num_devices=8,
                   num_swdge_queues=4)

    xT = nc.declare_dram_parameter("xT", [D, BT], bf16, isOutput=False)
    wa = nc.declare_dram_parameter("wa", [D, 128], bf16, isOutput=False)
    wv = nc.declare_dram_parameter("wv", [D, HD], bf16, isOutput=False)
    woh = nc.declare_dram_parameter("woh", [HD, D], bf16, isOutput=False)
    bqk = nc.declare_dram_parameter("bqk", [128, 1], f32, isOutput=False)
    bv = nc.declare_dram_parameter("bv", [HD, 1], f32, isOutput=False)
    outT = nc.declare_dram_parameter("outT", [NF, 128, BT], bf16, isOutput=True)
    den = nc.declare_dram_parameter("den", [1, BT], bf16, isOutput=True)

    # mask4[s, jj, t] = s <= t (same causal mask for each of 4 chunks)
    mask_np = np.broadcast_to(
        np.triu(np.ones((C, C), np.float32))[:, None, :], (C, 4, C)).copy()
    iden128_np = np.eye(C, dtype=ml_dtypes.bfloat16)
    mask_d = nc.inline_tensor(mask_np.reshape(C, 4 * C), "causal_mask4")
    iden128_d = nc.inline_tensor(iden128_np, "iden128")

    with tile.TileContext(nc) as tc:
        with (
            tc.tile_pool(name="consts", bufs=1) as consts,
        ):
            # ---- resident SBUF tensors (matmul operands in bf16) ----
            xt_sb = consts.tile([128, NF, BT], bf16)
            wa_sb = consts.tile([128, NF, 128], bf16)
            wv_sb = consts.tile([128, NF, HD], bf16)
            woh_sb = consts.tile([HD, D], bf16)
            bqk_sb = consts.tile([128, 1], f32)
            bv_sb = consts.tile([HD, 1], f32)
            mask_sb = consts.tile([C, 4, C], f32)
            iden128_sb = consts.tile([C, C], bf16)
            qk_phi = consts.tile([128, BT], bf16)      # rows 0-63 qT, 64-127 kT
            kvT = consts.tile([128, BT], bf16)         # rows 0-63 kT, 64-127 vT
            vT_sb = consts.tile([HD, BT], bf16)
            k_nrm = consts.tile([128, NCH, HD], bf16)
            v_aug = consts.tile([128, NCH, HD + 1], bf16)
            sm_all = consts.tile([C, NCH, C], bf16)
            Sb16 = consts.tile([HD, B, CPB - 1, HD + 1], bf16)
            onT_all = consts.tile([HD + 1, NCH, C], bf16)
            dumX = consts.tile([128, 512], bf16)

            # ---- input staging. Each dma_start blocks its queue ~600ns, so
            # the per-queue issue ORDER is the schedule: first-needed first.
            # (batched/rearranged weight DMAs explode into tiny descriptors
            # and delay queue-completion counts - keep tiles fine-grained)
            def xt_dma(eng, tcol, f):
                eng.dma_start(xt_sb[:, f, ts(tcol, 512)],
                              xT[128 * f:128 * (f + 1), ts(tcol, 512)])

            nc.sync.dma_start(wa_sb[:, 0, :], wa[0:128, :])
            nc.sync.dma_start(wa_sb[:, 1, :], wa[128:256, :])
            xt_dma(nc.sync, 0, 0)
            xt_dma(nc.sync, 0, 2)
            xt_dma(nc.gpsimd, 0, 1)
            nc.gpsimd.dma_start(wa_sb[:, 2, :], wa[256:384, :])
            nc.gpsimd.dma_start(wa_sb[:, 3, :], wa[384:512, :])
            nc.scalar.dma_start(bqk_sb[:], bqk[:, :])
            xt_dma(nc.scalar, 0, 3)
            nc.scalar.dma_start(bv_sb[:], bv[:, :])
            xt_dma(nc.scalar, 1, 1)
            xt_dma(nc.scalar, 1, 3)
            for f in range(NF):
                nc.gpsimd.dma_start(wv_sb[:, f, :], wv[128 * f:128 * (f + 1), :])
            nc.gpsimd.dma_start(iden128_sb[:], iden128_d[:, :])
            nc.gpsimd.dma_start(mask_sb[:],
                                mask_d.ap().rearrange("p (j t) -> p j t", j=4))
            nc.vector.memset(v_aug[:, :, HD:HD + 1], 1.0)
            nc.vector.memset(dumX[:], 0.0)
            xt_dma(nc.scalar, 2, 1)
            xt_dma(nc.scalar, 2, 3)
            xt_dma(nc.scalar, 3, 1)
            xt_dma(nc.scalar, 3, 3)
            for tcol in range(1, NT):
                xt_dma(nc.sync, tcol, 0)
                xt_dma(nc.sync, tcol, 2)
            nc.sync.dma_start(woh_sb[:], woh[:, :])

            with (
                tc.tile_pool(name="psA", bufs=3, space="PSUM") as psA,
                tc.tile_pool(name="psSc", bufs=2, space="PSUM") as psSc,
                tc.tile_pool(name="psT", bufs=3, space="PSUM") as psT,
                tc.tile_pool(name="ptmp", bufs=2) as ptmp,
                tc.tile_pool(name="attn", bufs=4) as attn,
            ):
                def proj_tile(tt):
                    sl = ts(tt, 512)
                    pa = psA.tile([128, 512], f32, tag="pa", name=f"pa{tt}")
                    pb = psA.tile([HD, 512], f32, tag="pa", name=f"pb{tt}")
                    for f in range(NF):
                        nc.tensor.matmul(pa, wa_sb[:, f, :], xt_sb[:, f, sl],
                                         start=(f == 0), stop=(f == NF - 1))
                        if tt <= 1:
                            filler(1)
                    for f in range(NF):
                        nc.tensor.matmul(pb, wv_sb[:, f, :], xt_sb[:, f, sl],
                                         start=(f == 0), stop=(f == NF - 1))
                        if tt <= 1:
                            filler(1)
                    if tt <= 1:
                        filler(2)
                    nc.scalar.activation(vT_sb[:, sl], pb, AF.Identity, bias=bv_sb[:])
                    # phi = exp(min(z,0)) + relu(z) = min(exp(z),1) + relu(z):
                    # both ACT ops read PSUM directly with fused bias
                    rr = ptmp.tile([128, 512], f32, tag="rr", name=f"rr{tt}")
                    ee = ptmp.tile([128, 512], f32, tag="ee", name=f"ee{tt}")
                    nc.scalar.activation(ee, pa, AF.Exp, bias=bqk_sb[:])
                    nc.scalar.activation(rr, pa, AF.Relu, bias=bqk_sb[:])
                    nc.vector.scalar_tensor_tensor(
                        qk_phi[:, sl], ee, 1.0, rr,
                        mybir.AluOpType.min, mybir.AluOpType.add)
                    # build [kT; vT] at base partition 0 (SBUF->SBUF DMAs):
                    # matmul operands must share a base partition, and the
                    # stacked tile transposes k and v chunks in ONE PE op
                    nc.sync.dma_start(kvT[0:HD, sl], qk_phi[64:128, sl])
                    nc.gpsimd.dma_start(kvT[HD:128, sl], vT_sb[:, sl])
                    # transposes + scores for the 4 chunks in this t-tile
                    filler(2)
                    ptr = psSc.tile([C, 4, C], bf16, tag="ps", name=f"tr{tt}")
                    psc = psSc.tile([C, 4, C], f32, tag="ps", name=f"ps{tt}")
                    for jj in range(4):
                        i = tt * 4 + jj
                        cs = ts(i, C)
                        nc.tensor.transpose(ptr[:, jj, :], kvT[:, cs],
                                            iden128_sb[:])
                        nc.tensor.matmul(psc[:, jj, :], kvT[0:HD, cs],
                                         qk_phi[0:64, cs], start=True, stop=True)
                    i0 = tt * 4
                    nc.vector.tensor_copy(k_nrm[:, i0:i0 + 4, :], ptr[:, :, 0:HD])
                    nc.vector.tensor_copy(v_aug[:, i0:i0 + 4, 0:HD],
                                          ptr[:, :, HD:2 * HD])
                    nc.vector.tensor_mul(sm_all[:, i0:i0 + 4, :], psc, mask_sb[:])

                def kv_part(b):
                    # kv outer products (PE) + bf16 prefix-state chain (DVE):
                    # Sb16[j] = Sb16[j-1] + pkv[j], j = 0..CPB-2
                    for w in range(2):
                        pkv = psT.tile([HD, 4, HD + 1], f32, tag="tr",
                                       name=f"pkv{b}{w}")
                        for jw in range(4 if w == 0 else 3):
                            j = 4 * w + jw
                            i = b * CPB + j
                            nc.tensor.matmul(pkv[:, jw, :], k_nrm[:, i, :],
                                             v_aug[:, i, :], start=True, stop=True)
                        for jw in range(4 if w == 0 else 3):
                            j = 4 * w + jw
                            if j == 0:
                                nc.vector.tensor_copy(Sb16[:, b, 0, :],
                                                      pkv[:, 0, :])
                            else:
                                nc.vector.tensor_add(Sb16[:, b, j, :],
                                                     Sb16[:, b, j - 1, :],
                                                     pkv[:, jw, :])

                poT = {}

                def po_pair(b, j):
                    # one chunk: intra + state matmuls ADJACENT (a start=True
                    # clears has_written for the whole bank, so a chunk's
                    # group must close before the next group in that bank
                    # opens). Chains of b=0 / b=1 are independent, so pairs
                    # are interleaved across batches to hide chain latency.
                    g = 2 * b + j // 4
                    jw = j % 4
                    i = b * CPB + j
                    if jw == 0:
                        poT[g] = psA.tile([HD + 1, 4, C], f32, tag="pa",
                                          name=f"poT{g}")
                    p = poT[g]
                    if j == 0:
                        nc.tensor.matmul(p[:, 0, :], v_aug[:, i, :],
                                         sm_all[:, i, :], start=True, stop=True)
                    else:
                        nc.tensor.matmul(p[:, jw, :], v_aug[:, i, :],
                                         sm_all[:, i, :], start=True, stop=False)
                        nc.tensor.matmul(p[:, jw, :], Sb16[:, b, j - 1, :],
                                         qk_phi[0:64, ts(i, C)],
                                         start=False, stop=True)

                def onT_copy(g):
                    b, w = g // 2, g % 2
                    i0 = b * CPB + 4 * w
                    if g == 3:
                        nc.vector.tensor_copy(onT_all[:, i0:i0 + 4, :],
                                              poT[g][:])
                    else:
                        nc.scalar.copy(onT_all[:, i0:i0 + 4, :], poT[g][:])

                def o_proj_mm(g, Dc):
                    i0 = (g // 2) * CPB + 4 * (g % 2)
                    ppT = psSc.tile([C, 512], f32, tag="ps", name=f"ppT{g}{Dc}")
                    nc.tensor.matmul(ppT, woh_sb[:, ts(Dc, 128)],
                                     onT_all[0:HD, i0:i0 + 4, :],
                                     start=True, stop=True)
                    osl = attn.tile([C, 512], bf16, tag="osl",
                                    name=f"osl{g}{Dc}")
                    # split the PSUM->SBUF bounce across both PSUM-capable
                    # engines so the pool slot frees in ~one half-copy; in
                    # the pairs phase (g0/g2) Vector is the pacer, so Scalar
                    # takes the whole tile there
                    if g % 2 == 0:
                        nc.scalar.copy(osl[:], ppT[:])
                    else:
                        nc.scalar.copy(osl[:, 0:256], ppT[:, 0:256])
                        nc.vector.tensor_copy(osl[:, 256:512],
                                              ppT[:, 256:512])
                    eng = (nc.sync, nc.gpsimd)[Dc % 2]
                    eng.dma_start(outT[Dc, :, ts(g, 512)], osl)

                dps = psA.tile([128, 512], f32, tag="pa", name="warm")
                dps2 = psT.tile([128, 512], f32, tag="tr", name="warm2")

                def filler(n):
                    # short always-ready matmuls: bridge dependency gaps so
                    # the PE p-state governor never sees a >150ns idle and
                    # keeps the array at max clock
                    for _ in range(n):
                        nc.tensor.matmul(dps2[:, 0:128], dumX[:, 0:128],
                                         dumX[:, 0:128], start=True, stop=True)

                for wi in range(8):
                    nc.tensor.matmul(dps, dumX[:, 0:128], dumX[:],
                                     start=True, stop=True)
                proj_tile(0)
                proj_tile(1)
                proj_tile(2)
                proj_tile(3)
                kv_part(0)
                kv_part(1)
                po_pair(0, 0)
                po_pair(0, 1)
                po_pair(0, 2)
                po_pair(0, 3)
                onT_copy(0)
                po_pair(0, 4)
                po_pair(0, 5)
                o_proj_mm(0, 0)
                o_proj_mm(0, 1)
                po_pair(0, 6)
                po_pair(0, 7)
                onT_copy(1)
                o_proj_mm(0, 2)
                o_proj_mm(0, 3)
                po_pair(1, 0)
                po_pair(1, 1)
                o_proj_mm(1, 0)
                o_proj_mm(1, 1)
                po_pair(1, 2)
                po_pair(1, 3)
                onT_copy(2)
                o_proj_mm(1, 2)
                o_proj_mm(1, 3)
                po_pair(1, 4)
                po_pair(1, 5)
                o_proj_mm(2, 0)
                o_proj_mm(2, 1)
                po_pair(1, 6)
                po_pair(1, 7)
                onT_copy(3)
                o_proj_mm(2, 2)
                o_proj_mm(2, 3)
                o_proj_mm(3, 0)
                o_proj_mm(3, 1)
                o_proj_mm(3, 2)
                o_proj_mm(3, 3)
                nc.sync.dma_start(den[0:1, :], onT_all[HD:HD + 1, :, :])

    nc.compile()
    _CACHE["nc"] = nc
    return nc


def _in_maps(x, Wq, bq, Wk, bk, Wv, bv, Wo, bo):
    import ml_dtypes
    bf = ml_dtypes.bfloat16
    x2 = np.ascontiguousarray(x.reshape(BT, D).T).astype(bf)
    WoT = np.ascontiguousarray(Wo.T)                  # [(h m), d]
    maps = []
    for c in range(8):
        sl = slice(HD * c, HD * (c + 1))
        maps.append(dict(
            xT=x2,
            wa=np.ascontiguousarray(np.concatenate([Wq[sl], Wk[sl]], 0).T).astype(bf),
            wv=np.ascontiguousarray(Wv[sl].T).astype(bf),
            woh=np.ascontiguousarray(WoT[sl]).astype(bf),
            bqk=np.ascontiguousarray(np.concatenate([bq[sl], bk[sl]]).reshape(128, 1)).astype(np.float32),
            bv=np.ascontiguousarray(bv[sl].reshape(HD, 1)).astype(np.float32),
        ))
    return maps


def kernel(x, Wq, bq, Wk, bk, Wv, bv, Wo, bo):
    from concourse import bass_utils

    nc = _build()
    maps = _in_maps(np.asarray(x), np.asarray(Wq), np.asarray(bq),
                    np.asarray(Wk), np.asarray(bk), np.asarray(Wv),
                    np.asarray(bv), np.asarray(Wo), np.asarray(bo))
    res = bass_utils.run_bass_kernel_spmd(nc, maps, core_ids=list(range(8)))
    # unshard: per-head UNNORMALIZED partials oT [D, BT] + denom [1, BT];
    # divide by each head's denominator, sum over heads, add bias
    accT = np.zeros((D, BT), np.float32)
    for c in range(8):
        oT = res.results[c]["outT"].astype(np.float32).reshape(D, BT)
        dn = np.maximum(res.results[c]["den"].astype(np.float32), 1e-6)
        accT += oT / dn
    o = accT.T.reshape(B, T, D) + np.asarray(bo).astype(np.float32)[None, None, :]
    return np.ascontiguousarray(o).astype(np.float32)


# revision 33
# speedup vs baseline: 1.0425x; 1.0405x over previous
"""Trainium2 Bass kernel for AssociativeMemoryModule (causal linear attention).

Sharding: head-parallel with output-partial unshard - core c owns head c for
both batches and NEVER communicates on-device (no collectives, no NRT
pre-collective barrier). Each core:
  1. projects full x (pre-transposed, bf16 on host) to [q.T;k.T] (128 rows)
     and v.T (64 rows); phi = min(exp(z),1) + relu(z) in f32 -> bf16,
  2. stacks [kT; vT] at base partition 0 so one PE transpose per 128-chunk
     yields both k and v in normal layout; masked scores batched per t-tile,
  3. chunked causal linear attention (C=128) computed TRANSPOSED: poT[m,t]
     needs no per-chunk PE transposes or epilogue copies since all operand
     layouts (v_aug [s,m], sm [s,t], Sb16 [d,m], qT [d,t]) already exist.
     The intra-chunk matmuls open PSUM accumulation groups early (filling
     the PE while the serial DVE state-chain runs); the state matmuls close
     them as each prefix state lands. State chain is kept in bf16 (one DVE
     add per chunk, no separate cast).
  4. o-projection flipped: oT[Dc,t] = woh[:,Dc].T @ onT, emitted UNNORMALIZED
     as bf16 [4,128,BT] plus the bf16 denominator row [1,BT] (row 64 of onT);
     the per-token 1/denom division commutes with o-proj and is done on host.
Host unshards: o = sum_h (oT_h / den_h).T + bo.
"""
import sys

import numpy as np

sys.path.insert(0, "/opt/trn_rl_repo")

H, HD, D = 8, 64, 512
B, T = 2, 1024
BT = B * T            # 2048
C = 128               # attention chunk
NCH = BT // C         # 16 chunks total
CPB = T // C          # 8 chunks per batch
NF = D // 128         # 4 feature tiles
NT = 4                # t-tiles of 512 for projections

_CACHE = {}


def _build():
    if "nc" in _CACHE:
        return _CACHE["nc"]
    import concourse.mybir as mybir
    import concourse.tile as tile
    from concourse import bacc
    from concourse.bass import ts

    import ml_dtypes

    f32 = mybir.dt.float32
    bf16 = mybir.dt.bfloat16
    AF = mybir.ActivationFunctionType

    nc = bacc.Bacc("TRN2", target_bir_lowering=False, debug=False, num_devices=8,
                   num_swdge_queues=4)

    xT = nc.declare_dram_parameter("xT", [D, BT], bf16, isOutput=False)
    wa = nc.declare_dram_parameter("wa", [D, 128], bf16, isOutput=False)
    wv = nc.declare_dram_parameter("wv", [D, HD], bf16, isOutput=False)
    woh = nc.declare_dram_parameter("woh", [HD, D], bf16, isOutput=False)
    bqk = nc.declare_dram_parameter("bqk", [128, 1], f32, isOutput=False)
    bv = nc.declare_dram_parameter("bv", [HD, 1], f32, isOutput=False)
    outT = nc.declare_dram_parameter("outT", [NF, 128, BT], bf16, isOutput=True)
    den = nc.declare_dram_parameter("den", [1, BT], bf16, isOutput=True)

    # mask4[s, jj, t] = s <= t (same causal mask for each of 4 chunks)
    mask_np = np.broadcast_to(
        np.triu(np.ones((C, C), np.float32))[:, None, :], (C, 4, C)).copy()
    iden128_np = np.eye(C, dtype=ml_dtypes.bfloat16)
    mask_d = nc.inline_tensor(mask_np.reshape(C, 4 * C), "causal_mask4")
    iden128_d = nc.inline_tensor(iden128_np, "iden128")

    with tile.TileContext(nc) as tc:
        with (
            tc.tile_pool(name="consts", bufs=1) as consts,
        ):
            # ---- resident SBUF tensors (matmul operands in bf16) ----
            xt_sb = consts.tile([128, NF, BT], bf16)
            wa_sb = consts.tile([128, NF, 128], bf16)
            wv_sb = consts.tile([128, NF, HD], bf16)
            woh_sb = consts.tile([HD, D], bf16)
            bqk_sb = consts.tile([128, 1], f32)
            bv_sb = consts.tile([HD, 1], f32)
            mask_sb = consts.tile([C, 4, C], f32)
            iden128_sb = consts.tile([C, C], bf16)
            qk_phi = consts.tile([128, BT], bf16)      # rows 0-63 qT, 64-127 kT
            kvT = consts.tile([128, BT], bf16)         # rows 0-63 kT, 64-127 vT
            vT_sb = consts.tile([HD, BT], bf16)
            k_nrm = consts.tile([128, NCH, HD], bf16)
            v_aug = consts.tile([128, NCH, HD + 1], bf16)
            sm_all = consts.tile([C, NCH, C], bf16)
            Sb16 = consts.tile([HD, B, CPB - 1, HD + 1], bf16)
            onT_all = consts.tile([HD + 1, NCH, C], bf16)
            dumX = consts.tile([128, 512], bf16)

            # ---- input staging. Each dma_start blocks its queue ~600ns, so
            # the per-queue issue ORDER is the schedule: first-needed first.
            # (batched/rearranged weight DMAs explode into tiny descriptors
            # and delay queue-completion counts - keep tiles fine-grained)
            def xt_dma(eng, tcol, f):
                eng.dma_start(xt_sb[:, f, ts(tcol, 512)],
                              xT[128 * f:128 * (f + 1), ts(tcol, 512)])

            nc.sync.dma_start(wa_sb[:, 0, :], wa[0:128, :])
            nc.sync.dma_start(wa_sb[:, 1, :], wa[128:256, :])
            xt_dma(nc.sync, 0, 0)
            xt_dma(nc.sync, 0, 2)
            xt_dma(nc.gpsimd, 0, 1)
            nc.gpsimd.dma_start(wa_sb[:, 2, :], wa[256:384, :])
            nc.gpsimd.dma_start(wa_sb[:, 3, :], wa[384:512, :])
            nc.scalar.dma_start(bqk_sb[:], bqk[:, :])
            xt_dma(nc.scalar, 0, 3)
            nc.scalar.dma_start(bv_sb[:], bv[:, :])
            xt_dma(nc.scalar, 1, 1)
            xt_dma(nc.scalar, 1, 3)
            for f in range(NF):
                nc.gpsimd.dma_start(wv_sb[:, f, :], wv[128 * f:128 * (f + 1), :])
            nc.gpsimd.dma_start(iden128_sb[:], iden128_d[:, :])
            nc.gpsimd.dma_start(mask_sb[:],
                                mask_d.ap().rearrange("p (j t) -> p j t", j=4))
            nc.vector.memset(v_aug[:, :, HD:HD + 1], 1.0)
            nc.vector.memset(dumX[:], 0.0)
            xt_dma(nc.scalar, 2, 1)
            xt_dma(nc.scalar, 2, 3)
            xt_dma(nc.scalar, 3, 1)
            xt_dma(nc.scalar, 3, 3)
            for tcol in range(1, NT):
                xt_dma(nc.sync, tcol, 0)
                xt_dma(nc.sync, tcol, 2)
            nc.sync.dma_start(woh_sb[:], woh[:, :])

            with (
                tc.tile_pool(name="psA", bufs=3, space="PSUM") as psA,
                tc.tile_pool(name="psSc", bufs=2, space="PSUM") as psSc,
                tc.tile_pool(name="psT", bufs=3, space="PSUM") as psT,
                tc.tile_pool(name="ptmp", bufs=2) as ptmp,
                tc.tile_pool(name="attn", bufs=4) as attn,
            ):
                def proj_tile(tt):
                    sl = ts(tt, 512)
                    pa = psA.tile([128, 512], f32, tag="pa", name=f"pa{tt}")
                    pb = psA.tile([HD, 512], f32, tag="pa", name=f"pb{tt}")
                    for f in range(NF):
                        nc.tensor.matmul(pa, wa_sb[:, f, :], xt_sb[:, f, sl],
                                         start=(f == 0), stop=(f == NF - 1))
                        if tt <= 1:
                            filler(1)
                    for f in range(NF):
                        nc.tensor.matmul(pb, wv_sb[:, f, :], xt_sb[:, f, sl],
                                         start=(f == 0), stop=(f == NF - 1))
                        if tt <= 1:
                            filler(1)
                    if tt <= 1:
                        filler(2)
                    nc.scalar.activation(vT_sb[:, sl], pb, AF.Identity, bias=bv_sb[:])
                    # phi = exp(min(z,0)) + relu(z) = min(exp(z),1) + relu(z):
                    # both ACT ops read PSUM directly with fused bias
                    rr = ptmp.tile([128, 512], f32, tag="rr", name=f"rr{tt}")
                    ee = ptmp.tile([128, 512], f32, tag="ee", name=f"ee{tt}")
                    nc.scalar.activation(ee, pa, AF.Exp, bias=bqk_sb[:])
                    nc.scalar.activation(rr, pa, AF.Relu, bias=bqk_sb[:])
                    nc.vector.scalar_tensor_tensor(
                        qk_phi[:, sl], ee, 1.0, rr,
                        mybir.AluOpType.min, mybir.AluOpType.add)
                    # build [kT; vT] at base partition 0 (SBUF->SBUF DMAs):
                    # matmul operands must share a base partition, and the
                    # stacked tile transposes k and v chunks in ONE PE op
                    nc.sync.dma_start(kvT[0:HD, sl], qk_phi[64:128, sl])
                    nc.gpsimd.dma_start(kvT[HD:128, sl], vT_sb[:, sl])
                    # transposes + scores for the 4 chunks in this t-tile
                    filler(2)
                    ptr = psSc.tile([C, 4, C], bf16, tag="ps", name=f"tr{tt}")
                    psc = psSc.tile([C, 4, C], f32, tag="ps", name=f"ps{tt}")
                    for jj in range(4):
                        i = tt * 4 + jj
                        cs = ts(i, C)
                        nc.tensor.transpose(ptr[:, jj, :], kvT[:, cs],
                                            iden128_sb[:])
                        nc.tensor.matmul(psc[:, jj, :], kvT[0:HD, cs],
                                         qk_phi[0:64, cs], start=True, stop=True)
                    i0 = tt * 4
                    nc.vector.tensor_copy(k_nrm[:, i0:i0 + 4, :], ptr[:, :, 0:HD])
                    nc.vector.tensor_copy(v_aug[:, i0:i0 + 4, 0:HD],
                                          ptr[:, :, HD:2 * HD])
                    nc.vector.tensor_mul(sm_all[:, i0:i0 + 4, :], psc, mask_sb[:])

                def kv_part(b):
                    # kv outer products (PE) + bf16 prefix-state chain (DVE):
                    # Sb16[j] = Sb16[j-1] + pkv[j], j = 0..CPB-2
                    for w in range(2):
                        pkv = psT.tile([HD, 4, HD + 1], f32, tag="tr",
                                       name=f"pkv{b}{w}")
                        for jw in range(4 if w == 0 else 3):
                            j = 4 * w + jw
                            i = b * CPB + j
                            nc.tensor.matmul(pkv[:, jw, :], k_nrm[:, i, :],
                                             v_aug[:, i, :], start=True, stop=True)
                        for jw in range(4 if w == 0 else 3):
                            j = 4 * w + jw
                            if j == 0:
                                nc.vector.tensor_copy(Sb16[:, b, 0, :],
                                                      pkv[:, 0, :])
                            else:
                                nc.vector.tensor_add(Sb16[:, b, j, :],
                                                     Sb16[:, b, j - 1, :],
                                                     pkv[:, jw, :])

                poT = {}

                def po_pair(b, j):
                    # one chunk: intra + state matmuls ADJACENT (a start=True
                    # clears has_written for the whole bank, so a chunk's
                    # group must close before the next group in that bank
                    # opens). Chains of b=0 / b=1 are independent, so pairs
                    # are interleaved across batches to hide chain latency.
                    g = 2 * b + j // 4
                    jw = j % 4
                    i = b * CPB + j
                    if jw == 0:
                        poT[g] = psA.tile([HD + 1, 4, C], f32, tag="pa",
                                          name=f"poT{g}")
                    p = poT[g]
                    if j == 0:
                        nc.tensor.matmul(p[:, 0, :], v_aug[:, i, :],
                                         sm_all[:, i, :], start=True, stop=True)
                    else:
                        nc.tensor.matmul(p[:, jw, :], v_aug[:, i, :],
                                         sm_all[:, i, :], start=True, stop=False)
                        nc.tensor.matmul(p[:, jw, :], Sb16[:, b, j - 1, :],
                                         qk_phi[0:64, ts(i, C)],
                                         start=False, stop=True)

                def onT_copy(g):
                    b, w = g // 2, g % 2
                    i0 = b * CPB + 4 * w
                    if g == 3:
                        nc.vector.tensor_copy(onT_all[:, i0:i0 + 4, :],
                                              poT[g][:])
                    else:
                        nc.scalar.copy(onT_all[:, i0:i0 + 4, :], poT[g][:])

                def o_proj_mm(g, Dc):
                    i0 = (g // 2) * CPB + 4 * (g % 2)
                    ppT = psSc.tile([C, 512], f32, tag="ps", name=f"ppT{g}{Dc}")
                    nc.tensor.matmul(ppT, woh_sb[:, ts(Dc, 128)],
                                     onT_all[0:HD, i0:i0 + 4, :],
                                     start=True, stop=True)
                    osl = attn.tile([C, 512], bf16, tag="osl",
                                    name=f"osl{g}{Dc}")
                    # split the PSUM->SBUF bounce across both PSUM-capable
                    # engines so the pool slot frees in ~one half-copy; in
                    # the pairs phase (g0/g2) Vector is the pacer, so Scalar
                    # takes the whole tile there
                    if g % 2 == 0:
                        nc.scalar.copy(osl[:], ppT[:])
                    else:
                        nc.scalar.copy(osl[:, 0:256], ppT[:, 0:256])
                        nc.vector.tensor_copy(osl[:, 256:512],
                                              ppT[:, 256:512])
                    eng = (nc.sync, nc.gpsimd)[Dc % 2]
                    eng.dma_start(outT[Dc, :, ts(g, 512)], osl)

                dps = psA.tile([128, 512], f32, tag="pa", name="warm")
                dps2 = psT.tile([128, 512], f32, tag="tr", name="warm2")

                def filler(n):
                    # short always-ready matmuls: bridge dependency gaps so
                    # the PE p-state governor never sees a >150ns idle and
                    # keeps the array at max clock
                    for _ in range(n):
                        nc.tensor.matmul(dps2[:, 0:128], dumX[:, 0:128],
                                         dumX[:, 0:128], start=True, stop=True)

                for wi in range(7):
                    nc.tensor.matmul(dps, dumX[:, 0:128], dumX[:],
                                     start=True, stop=True)
                proj_tile(0)
                proj_tile(1)
                proj_tile(2)
                proj_tile(3)
                kv_part(0)
                kv_part(1)
                po_pair(0, 0)
                po_pair(0, 1)
                po_pair(0, 2)
                po_pair(0, 3)
                onT_copy(0)
                po_pair(0, 4)
                po_pair(0, 5)
                o_proj_mm(0, 0)
                o_proj_mm(0, 1)
                po_pair(0, 6)
                po_pair(0, 7)
                onT_copy(1)
                o_proj_mm(0, 2)
                o_proj_mm(0, 3)
                po_pair(1, 0)
                po_pair(1, 1)
                o_proj_mm(1, 0)
                o_proj_mm(1, 1)
                po_pair(1, 2)
                po_pair(1, 3)
                onT_copy(2)
                o_proj_mm(1, 2)
                o_proj_mm(1, 3)
                po_pair(1, 4)
                po_pair(1, 5)
                o_proj_mm(2, 0)
                o_proj_mm(2, 1)
                po_pair(1, 6)
                po_pair(1, 7)
                onT_copy(3)
                o_proj_mm(2, 2)
                o_proj_mm(2, 3)
                o_proj_mm(3, 0)
                o_proj_mm(3, 1)
                o_proj_mm(3, 2)
                o_proj_mm(3, 3)
                nc.sync.dma_start(den[0:1, :], onT_all[HD:HD + 1, :, :])

    nc.compile()
    _CACHE["nc"] = nc
    return nc


def _in_maps(x, Wq, bq, Wk, bk, Wv, bv, Wo, bo):
    import ml_dtypes
    bf = ml_dtypes.bfloat16
    x2 = np.ascontiguousarray(x.reshape(BT, D).T).astype(bf)
    WoT = np.ascontiguousarray(Wo.T)                  # [(h m), d]
    maps = []
    for c in range(8):
        sl = slice(HD * c, HD * (c + 1))
        maps.append(dict(
            xT=x2,
            wa=np.ascontiguousarray(np.concatenate([Wq[sl], Wk[sl]], 0).T).astype(bf),
            wv=np.ascontiguousarray(Wv[sl].T).astype(bf),
            woh=np.ascontiguousarray(WoT[sl]).astype(bf),
            bqk=np.ascontiguousarray(np.concatenate([bq[sl], bk[sl]]).reshape(128, 1)).astype(np.float32),
            bv=np.ascontiguousarray(bv[sl].reshape(HD, 1)).astype(np.float32),
        ))
    return maps


def kernel(x, Wq, bq, Wk, bk, Wv, bv, Wo, bo):
    from concourse import bass_utils

    nc = _build()
    maps = _in_maps(np.asarray(x), np.asarray(Wq), np.asarray(bq),
                    np.asarray(Wk), np.asarray(bk), np.asarray(Wv),
                    np.asarray(bv), np.asarray(Wo), np.asarray(bo))
    res = bass_utils.run_bass_kernel_spmd(nc, maps, core_ids=list(range(8)))
    # unshard: per-head UNNORMALIZED partials oT [D, BT] + denom [1, BT];
    # divide by each head's denominator, sum over heads, add bias
    accT = np.zeros((D, BT), np.float32)
    for c in range(8):
        oT = res.results[c]["outT"].astype(np.float32).reshape(D, BT)
        dn = np.maximum(res.results[c]["den"].astype(np.float32), 1e-6)
        accT += oT / dn
    o = accT.T.reshape(B, T, D) + np.asarray(bo).astype(np.float32)[None, None, :]
    return np.ascontiguousarray(o).astype(np.float32)
